# revision 5
# baseline (speedup 1.0000x reference)
# Trainium2 Bass kernel v2 for LN->QKV->sparse-rel-pos-attention->proj.
#
# Baseline (kernel.py) measured 599us, PE busy 500us of which ~348us is
# unavoidable column streaming. v2 targets the gap:
#   - fp16 matmul operands everywhere (FWL weight loads, 2x DVE on fp16 ops)
#   - 6-chunk qk packing (q:0-2, k:3-5, strip h%4) removes the 25% padding
#     waste of the 8-chunk layout; head pairs (2g,2g+1) always mix strips,
#     and each PSUM bank holds a single head so tile_position interleaving
#     stays bank-pure (the hang documented in v1 was same-bank interleave).
#   - software-pipelined attention: S(g+2) + exp + expB-mult are emitted two
#     groups ahead of Z(g)/AV(g) so ACT/DVE feeder latency hides behind PE
#     work; prologue S(0),S(1) issued before the v-phase.
#   - startup: pair-0 x DMAs issued before the big weight DMAs (v1 stalled
#     43us waiting for x behind 11.5MB of weights).
#   - merged per-group score tile [128,(hl,mc,n)=1024] -> single exp, single
#     fp16 expB multiply.
#   - Z row-sum matmul accumulates into a dedicated 1-bank PSUM tile;
#     proj y accumulator is carved from the score pool (PSUM: 2+4+1+1 = 8).
import os
import numpy as np

B, N, DIM = 128, 256, 512
H, KD = 12, 32
D = 128
DH = D * H
RES = 16
EPS = 1e-5
NCORES = 8
BPC = B // NCORES
NCHUNK = 6  # q chunks 0-2, k chunks 3-5

_CACHE = {}


def _build(bpc, use_bqk, use_bp, merged_exp=True, norm_mode="alt"):
    from contextlib import ExitStack

    import concourse.bacc as bacc
    import concourse.tile as tile
    from concourse import mybir
    from concourse.masks import make_identity

    f32 = mybir.dt.float32
    f16 = mybir.dt.float16
    Alu = mybir.AluOpType
    Act = mybir.ActivationFunctionType

    nc = bacc.Bacc("TRN2", target_bir_lowering=False, debug=False,
                   num_devices=NCORES)

    x_d = nc.dram_tensor("x", [bpc, N, DIM], f32, kind="ExternalInput").ap()
    wqk_d = nc.dram_tensor("wqk", [DIM, NCHUNK * 128], f16,
                           kind="ExternalInput").ap()
    wv_d = nc.dram_tensor("wv", [DIM, DH], f16, kind="ExternalInput").ap()
    wp_d = nc.dram_tensor("wp", [DH, DIM], f16, kind="ExternalInput").ap()
    expb_d = nc.dram_tensor("expb", [128, H, 2, N], f16,
                            kind="ExternalInput").ap()
    ones_d = nc.dram_tensor("ones", [128, 1], f16, kind="ExternalInput").ap()
    if use_bqk:
        bqk_d = nc.dram_tensor("bqk", [128, NCHUNK], f32,
                               kind="ExternalInput").ap()
    if use_bp:
        bp_d = nc.dram_tensor("bp", [DIM], f32, kind="ExternalInput").ap()
    y_d = nc.dram_tensor("y", [bpc, N, DIM], f32, kind="ExternalOutput").ap()

    with tile.TileContext(nc) as tc, ExitStack() as ctx:
        consts = ctx.enter_context(tc.tile_pool(name="consts", bufs=1))
        sb_x = ctx.enter_context(tc.tile_pool(name="sb_x", bufs=8))
        sb_x16 = ctx.enter_context(tc.tile_pool(name="sb_x16", bufs=2))
        sb_zT = ctx.enter_context(tc.tile_pool(name="sb_zT", bufs=2))
        sb_qkT = ctx.enter_context(tc.tile_pool(name="sb_qkT", bufs=2))
        sb_v = ctx.enter_context(tc.tile_pool(name="sb_v", bufs=2))
        sb_pt = ctx.enter_context(tc.tile_pool(name="sb_pt", bufs=4))
        sb_zb = ctx.enter_context(tc.tile_pool(name="sb_zb", bufs=4))
        sb_zr = ctx.enter_context(tc.tile_pool(name="sb_zr", bufs=2))
        sb_ot = ctx.enter_context(tc.tile_pool(name="sb_ot", bufs=2))
        sb_yb = ctx.enter_context(tc.tile_pool(name="sb_yb", bufs=2))
        sb_small = ctx.enter_context(tc.tile_pool(name="sb_small", bufs=3))
        ps_work = ctx.enter_context(tc.tile_pool(name="ps_work", bufs=2,
                                                 space="PSUM"))
        ps_s = ctx.enter_context(tc.tile_pool(name="ps_s", bufs=2,
                                              space="PSUM"))
        ps_ot = ctx.enter_context(tc.tile_pool(name="ps_ot", bufs=2,
                                               space="PSUM"))
        dram = ctx.enter_context(tc.tile_pool(name="dram", bufs=3,
                                              space="DRAM"))

        # ---- constants; x for pair 0 is DMA'd FIRST so LN can start while
        # the big weight tensors stream in.
        ident = consts.tile([128, 128], f16)
        make_identity(nc, ident)
        eps_t = consts.tile([128, 1], f32)
        nc.vector.memset(eps_t, EPS)

        xt = {}

        def fetch_pair(ep):
            for el in range(2):
                for tci in range(2):
                    x_t = sb_x.tile([128, DIM], f32, tag="x")
                    nc.sync.dma_start(
                        out=x_t,
                        in_=x_d[2 * ep + el, tci * 128:(tci + 1) * 128, :])
                    xt[(ep, el, tci)] = x_t

        fetch_pair(0)

        wqk_sb = consts.tile([128, 4, NCHUNK * 128], f16)
        nc.sync.dma_start(out=wqk_sb,
                          in_=wqk_d.rearrange("(kc p) f -> p kc f", p=128))
        expb_sb = consts.tile([128, H, 2, N], f16)
        nc.sync.dma_start(out=expb_sb, in_=expb_d)
        wv_sb = consts.tile([128, 4, DH], f16)
        nc.sync.dma_start(out=wv_sb,
                          in_=wv_d.rearrange("(kc p) f -> p kc f", p=128))
        wp_sb = consts.tile([128, H, DIM], f16)
        nc.sync.dma_start(out=wp_sb,
                          in_=wp_d.rearrange("(h p) f -> p h f", p=128))
        ones_col = consts.tile([128, 1], f16)
        nc.sync.dma_start(out=ones_col, in_=ones_d)
        if use_bqk:
            bqk_sb = consts.tile([128, NCHUNK], f32)
            nc.sync.dma_start(out=bqk_sb, in_=bqk_d)
        if use_bp:
            bp_sb = consts.tile([128, 1, DIM], f32)
            nc.sync.dma_start(out=bp_sb, in_=bp_d.partition_broadcast(128))

        # ---- PE warmup: dense dummy matmuls while the first x tiles are in
        # flight, so the HAM clock gate reaches 2.4 GHz before real work
        # (otherwise everything until ~24us runs at 1.2 GHz).
        for wi in range(3):
            warm_ps = ps_work.tile([128, 512], f32, tag="work")
            for wj in range(24):
                nc.tensor.matmul(warm_ps[:, (wj % 4) * 128:(wj % 4) * 128 + 128],
                                 lhsT=ident, rhs=ident,
                                 start=True, stop=True)

        assert bpc % 2 == 0
        npair = bpc // 2
        for ep in range(npair):
            if ep + 1 < npair:
                fetch_pair(ep + 1)

            # ---- LayerNorm (token-major); all four bn-stats first so ACT
            # runs a single Sqrt per pair (table reloads cost ~1.3us).
            zT_sb = sb_zT.tile([128, 4, 2 * N], f16, tag="zT")
            mv = sb_small.tile([128, 2, 2, 2], f32, tag="mv")
            for el in range(2):
                for tci in range(2):
                    x_t = xt[(ep, el, tci)]
                    stats = sb_small.tile([128, 6], f32, tag="stats")
                    nc.vector.bn_stats(stats, x_t)
                    nc.vector.bn_aggr(mv[:, el, tci, :], stats)
            sig = sb_small.tile([128, 2, 2], f32, tag="sig")
            nc.scalar.activation(sig, mv[:, :, :, 1], Act.Sqrt, bias=eps_t,
                                 scale=1.0)
            rsig = sb_small.tile([128, 2, 2], f32, tag="rsig")
            nc.vector.reciprocal(rsig, sig)
            for el in range(2):
                for tci in range(2):
                    x_t = xt.pop((ep, el, tci))
                    x16 = sb_x16.tile([128, DIM], f16, tag="x16")
                    nc.vector.tensor_scalar(out=x16, in0=x_t,
                                            scalar1=mv[:, el, tci, 0:1],
                                            scalar2=rsig[:, el, tci:tci + 1],
                                            op0=Alu.subtract, op1=Alu.mult)
                    zT_ps = ps_work.tile([128, 512], f16, tag="work")
                    for kc in range(4):
                        nc.tensor.transpose(zT_ps[:, kc * 128:(kc + 1) * 128],
                                            x16[:, kc * 128:(kc + 1) * 128],
                                            ident)
                    off = el * N + tci * 128
                    nc.vector.tensor_copy(
                        out=zT_sb[:, :, off:off + 128],
                        in_=zT_ps.rearrange("p (kc t) -> p kc t", kc=4))

            # ---- qk^T = W'' ^T z^T  [feat, tok-pair]; head h: q in chunk
            # h//4, k in chunk 3+h//4, both at 32-row strip h%4.
            qkT_sb = sb_qkT.tile([128, NCHUNK, 2 * N], f16, tag="qkT")
            for fc in (0, 3, 1, 4, 2, 5):
                qk_ps = ps_work.tile([128, 512], f32, tag="work")
                for kc in range(4):
                    nc.tensor.matmul(qk_ps,
                                     lhsT=wqk_sb[:, kc, fc * 128:(fc + 1) * 128],
                                     rhs=zT_sb[:, kc, :],
                                     start=(kc == 0), stop=(kc == 3))
                nc.scalar.activation(qkT_sb[:, fc, :], qk_ps, Act.Copy)
                if use_bqk:
                    nc.vector.tensor_scalar_add(
                        out=qkT_sb[:, fc, :], in0=qkT_sb[:, fc, :],
                        scalar1=bqk_sb[:, fc:fc + 1])

            for el in range(2):
                e = 2 * ep + el
                etok = el * N

                # ---- attention pipeline state; score tile layout is
                # [m-part, (hl, mc, n)] = [128, 1024]; each PSUM bank holds a
                # single head's scores.
                pts = {}
                spss = {}

                def emit_S(g):
                    s_ps = ps_s.tile([128, 1024], f32, tag="s")
                    for mc in range(2):
                        for hl in range(2):
                            h = 2 * g + hl
                            qc = h // 4
                            base = (h % 4) * KD
                            nc.tensor.matmul(
                                s_ps[:, hl * 512 + mc * 256:
                                     hl * 512 + mc * 256 + 256],
                                lhsT=qkT_sb[base:base + KD, 3 + qc,
                                            etok + mc * 128:
                                            etok + (mc + 1) * 128],
                                rhs=qkT_sb[base:base + KD, qc,
                                           etok:etok + N],
                                start=True, stop=True,
                                tile_position=(base, 0))
                    pt = sb_pt.tile([128, 1024], f16, tag="pt")
                    # exp and expB-mult split into hl halves pipelined across
                    # ACT and GpSimd; the DVE carries only recip+norm.
                    for hl in range(2):
                        nc.scalar.activation(
                            pt[:, hl * 512:(hl + 1) * 512],
                            s_ps[:, hl * 512:(hl + 1) * 512], Act.Exp)
                        eng = nc.gpsimd
                        eng.tensor_tensor(
                            out=pt[:, hl * 512:(hl + 1) * 512],
                            in0=pt[:, hl * 512:(hl + 1) * 512],
                            in1=expb_sb[:, 2 * g + hl, :, :].rearrange(
                                "p b n -> p (b n)"),
                            op=Alu.mult)
                    pts[g] = pt
                    spss[g] = s_ps

                # ---- v = z Wv  [tok 256, feat 1536]; rounds are interleaved
                # into the attention loop so the PE FIFO always has
                # independent matmuls between dependent attention ones.
                # AV(g) reads head features [2g*128, (2g+2)*128) -> needs v
                # round ns=g//2; rounds (mc,ns) emitted >=2 groups ahead.
                v_sb = sb_v.tile([128, 2, DH], f16, tag="v")

                def vmm(mc, ns):
                    v_ps = ps_work.tile([128, 512], f32, tag="work")
                    for kc in range(4):
                        nc.tensor.matmul(
                            v_ps,
                            lhsT=zT_sb[:, kc,
                                       etok + mc * 128:etok + (mc + 1) * 128],
                            rhs=wv_sb[:, kc, ns * 512:(ns + 1) * 512],
                            start=(kc == 0), stop=(kc == 3))
                    return v_ps

                def vcopy(v_ps, mc, ns):
                    # GpSimd cannot read PSUM; copies go to ACT.
                    nc.scalar.activation(
                        v_sb[:, mc, ns * 512:(ns + 1) * 512], v_ps, Act.Copy)

                def vround(mc, ns):
                    vcopy(vmm(mc, ns), mc, ns)

                vround(0, 0)
                vround(1, 0)
                emit_S(0)
                emit_S(1)
                vround(0, 1)
                vsched = {0: (1, 1), 1: (0, 2), 2: (1, 2)}

                # ---- attention main loop, 2-group S lookahead; normalize
                # deferred one group so the zb DMA roundtrip never blocks
                # the DVE queue head.
                ot_sb = sb_ot.tile([128, H, N], f16, tag="ot")
                otps = {}
                zbs = {}

                def emit_norm(g):
                    nc.vector.tensor_tensor(
                        out=ot_sb[:, 2 * g:2 * g + 2, :],
                        in0=otps.pop(g).rearrange("p (a n) -> p a n", a=2),
                        in1=zbs.pop(g), op=Alu.mult)

                for g in range(6):
                    if g + 2 < 6:
                        emit_S(g + 2)
                    if g in vsched:
                        vround(*vsched[g])
                    pt = pts.pop(g)
                    pt_r = pt.rearrange("p (a b n) -> p a b n", a=2, b=2)
                    # Z[hl,n] = sum_m P^T[m, (hl,n)], split per hl so half 0
                    # starts as soon as mult(hl=0) lands. For g>=4 the row is
                    # carved from this group's own score tile (no future S
                    # reuses it); earlier groups use a work-pool bank so the
                    # next-pair qkv never waits on the el-tail z chain.
                    if g >= 4:
                        zrow = spss.pop(g)[0:1, 0:512]
                    else:
                        spss.pop(g)
                        zrow = ps_work.tile([1, 512], f32, tag="work")
                    for hl in range(2):
                        for mc in range(2):
                            nc.tensor.matmul(zrow[:, hl * N:(hl + 1) * N],
                                             lhsT=ones_col,
                                             rhs=pt_r[:, hl, mc, :],
                                             start=(mc == 0), stop=(mc == 1))
                    zrecip = sb_zr.tile([1, 512], f32, tag="zrecip")
                    nc.vector.reciprocal_approx_fast(out=zrecip, in_=zrow)
                    zscr = dram.tile([1, 512], f32, tag="zscr")
                    nc.sync.dma_start(out=zscr, in_=zrecip)
                    zb_sb = sb_zb.tile([128, 2, N], f32, tag="zb")
                    nc.sync.dma_start(out=zb_sb,
                                      in_=zscr[0, :].partition_broadcast(128))
                    zbs[g] = zb_sb
                    # O^T = v^T P^T
                    ot_ps = ps_ot.tile([128, 512], f32, tag="otp")
                    for hl in range(2):
                        h = 2 * g + hl
                        for mc in range(2):
                            nc.tensor.matmul(
                                ot_ps[:, hl * N:(hl + 1) * N],
                                lhsT=v_sb[:, mc, h * 128:(h + 1) * 128],
                                rhs=pt_r[:, hl, mc, :],
                                start=(mc == 0), stop=(mc == 1))
                    otps[g] = ot_ps
                    if g >= 1:
                        emit_norm(g - 1)
                emit_norm(5)

                # ---- proj: y = O Wp; accumulator carved from the score pool
                y_ps = ps_s.tile([128, 1024], f32, tag="s")
                for nci in range(2):
                    yr = y_ps[:, nci * 512:(nci + 1) * 512]
                    for h in range(H):
                        nc.tensor.matmul(
                            yr,
                            lhsT=ot_sb[:, h, nci * 128:(nci + 1) * 128],
                            rhs=wp_sb[:, h, :],
                            start=(h == 0), stop=(h == H - 1))
                    yb_sb = sb_yb.tile([128, DIM], f32, tag="yb")
                    if use_bp:
                        nc.vector.tensor_tensor(out=yb_sb, in0=yr,
                                                in1=bp_sb[:, 0, :],
                                                op=Alu.add)
                    else:
                        nc.scalar.activation(yb_sb, yr, Act.Copy)
                    nc.sync.dma_start(out=y_d[e, nci * 128:(nci + 1) * 128, :],
                                      in_=yb_sb)

    nc.compile()
    return nc


def _prepare(x, gamma, beta, Wqkv, bqkv, Wproj, bproj, biases, bias_idxs):
    x = np.ascontiguousarray(np.asarray(x, dtype=np.float32))
    gamma = np.asarray(gamma, dtype=np.float32)
    beta = np.asarray(beta, dtype=np.float32)
    Wqkv = np.asarray(Wqkv, dtype=np.float32)
    bqkv = np.asarray(bqkv, dtype=np.float32)
    Wproj = np.asarray(Wproj, dtype=np.float32)
    bproj = np.asarray(bproj, dtype=np.float32)
    biases = np.asarray(biases, dtype=np.float32)
    bias_idxs = np.asarray(bias_idxs)

    s = np.float32(KD ** -0.5)
    Wg = Wqkv * gamma[:, None]
    bfull = beta @ Wqkv + bqkv
    Wr = Wg.reshape(DIM, H, 64 + D)
    br = bfull.reshape(H, 64 + D)
    # head h -> strip h%4; q in chunk h//4, k in chunk 3 + h//4.
    wqk = np.zeros((DIM, NCHUNK, 128), dtype=np.float32)
    bqk = np.zeros((NCHUNK, 128), dtype=np.float32)
    for h in range(H):
        qc, base = h // 4, (h % 4) * KD
        wqk[:, qc, base:base + KD] = Wr[:, h, 0:KD] * s
        wqk[:, 3 + qc, base:base + KD] = Wr[:, h, KD:2 * KD]
        bqk[qc, base:base + KD] = br[h, 0:KD] * s
        bqk[3 + qc, base:base + KD] = br[h, KD:2 * KD]
    wqk = np.ascontiguousarray(wqk.reshape(DIM, NCHUNK * 128), dtype=np.float16)
    wv = np.ascontiguousarray(Wr[:, :, 2 * KD:].reshape(DIM, DH),
                              dtype=np.float16)
    bv = br[:, 2 * KD:].reshape(DH)
    bp = bproj + bv @ Wproj
    expb = np.exp(biases[:, bias_idxs])  # [H, N, N]
    # [m-part, h, mc, n]
    expb_t = np.ascontiguousarray(
        expb.reshape(H, 2, 128, N).transpose(2, 0, 1, 3), dtype=np.float16)

    use_bqk = bool(np.abs(bqk).max() > 0)
    use_bp = bool(np.abs(bp).max() > 0)
    bqk_t = np.ascontiguousarray(bqk.T, dtype=np.float32)  # [128, NCHUNK]

    common = {"wqk": wqk, "wv": wv,
              "wp": np.ascontiguousarray(Wproj, dtype=np.float16),
              "expb": expb_t, "ones": np.ones((128, 1), dtype=np.float16)}
    if use_bqk:
        common["bqk"] = bqk_t
    if use_bp:
        common["bp"] = np.ascontiguousarray(bp, dtype=np.float32)
    in_maps = []
    for c in range(NCORES):
        m = dict(common)
        m["x"] = np.ascontiguousarray(x[c * BPC:(c + 1) * BPC])
        in_maps.append(m)
    return in_maps, use_bqk, use_bp


def run(inputs, trace=False, merged_exp=True, norm_mode="alt", **run_kwargs):
    from concourse.bass_utils import run_bass_kernel_spmd

    in_maps, use_bqk, use_bp = _prepare(**inputs)
    key = (BPC, use_bqk, use_bp, merged_exp, norm_mode)
    if key not in _CACHE:
        _CACHE[key] = _build(*key)
    nc = _CACHE[key]
    res = run_bass_kernel_spmd(nc, in_maps, core_ids=list(range(NCORES)),
                               trace=trace, **run_kwargs)
    y = np.concatenate([res.results[c]["y"] for c in range(NCORES)], axis=0)
    return y, res


def kernel(**inputs):
    y, _ = run(inputs)
    return y
